# revision 30
# baseline (speedup 1.0000x reference)
"""AffectiveMemoryUnit1D fused Trainium2 kernel (bf16 pipeline).

Math (per batch element, fully fused; weights pre-collapsed on host):
    z^T  = W_ag @ xs^T                   xs = (x - mean_d x) * rsqrt(var_d x + eps)
    e    = exp(z + c2 - C),  Z_k = sum_n e        (constant shift C: softmax
                                                   is shift-invariant; verified
                                                   no overflow for this regime)
    h3_un = (e invZ) @ W_b.T             (1/S normalization deferred)
    out  = relu(h3_un * s* + b* + x)
  where LN-2 cancels the 1/S scale except inside the eps regularizer:
    s* = 1/sqrt(q - p^2 + eps*(S+1e-9)^2),  b* = -p*s*
    q  = sumsq_d(h3_un)/D = e^T G e / D   with G = Wbs @ Wbs^T (Gram)
    p  = sum_d(h3_un)/D   = e . (Wbs rowsums)/D
    S  = e . invZ
  The residual x is accumulated into the h3 PSUM tile by an extra
  diag(1/s*)-weighted matmul; the epilogue is a single fused
  Relu(psum * s* + b*) (ACT) or s* * max(psum, 0) with b* pre-folded into
  the residual rhs (DVE/Pool), selected per tile to balance engines.

Dataflow: x and out travel as bf16 (host converts), halving HBM traffic.
Sharding: data-parallel over B=8, one batch element per NeuronCore.
"""

import numpy as np
from contextlib import ExitStack

import concourse.bass as bass
import concourse.tile as tile
from concourse import bacc, mybir
from concourse.bass_utils import run_bass_kernel_spmd
from concourse.masks import make_identity

F32 = mybir.dt.float32
BF16 = mybir.dt.bfloat16
AF = mybir.ActivationFunctionType
OP = mybir.AluOpType
AX = mybir.AxisListType

B, N, D, K = 8, 4096, 1024, 128
LN_EPS = 1e-5
C_SHIFT = 16.0         # constant softmax shift (replaces max subtraction)
NT = N // 128          # 32 token tiles of (128, D)
DC = D // 128          # 8 contraction chunks
WV = 4                 # tiles per phase-A wave == tiles per z-group
NW = NT // WV          # 8 waves / z-groups
GS = 4                 # tiles per phase-B group
NG = NT // GS          # 8 groups

# ---- engine schedules (tunable). 'A' = ACT, 'D' = DVE, 'P' = Pool/gpsimd,
# 'M' = DMA, 'V' = DVE max0*s epilogue variant. GPSIMD cannot touch PSUM, so
# Pool only gets SBUF->SBUF work (its share of the LN stats).
# per-tile stats path: D = bn_stats pair, A = ACT accum passes (Pool cannot
# do free-axis reductions)
STAT_ENG = (['D'] * 7 + ['A']) * 4
# per-tile xr = x*r pre-scale (mean fix rides the rank-1 matmuls instead)
XR_ENG = ['P', 'D', 'P', 'P', 'D', 'P', 'P', 'D'] * 4
# PSUM->SBUF transpose-evict half-tile units, 8 per wave (4 tiles x 2 halves)
EV_ENG = (['A', 'A', 'D', 'A', 'A', 'D', 'A', 'D'] * 4 +
          ['A', 'A', 'D', 'A', 'A', 'D', 'A', 'A'] * 4)
# epilogue engine per tile (ACT fused relu / DVE max0*mult)
EPI_ENG = ['A', 'A', 'V', 'A', 'A', 'A', 'V', 'A'] * 4

_CACHE = {}


def _build(reps=1):
    nc = bacc.Bacc(dynamic_dma_scratch_size=2048)

    x_d = nc.dram_tensor("x", [N, D], BF16, kind="ExternalInput")
    wagT_d = nc.dram_tensor("w_agT", [D, K], BF16, kind="ExternalInput")
    wbT_d = nc.dram_tensor("w_bT", [K, D], F32, kind="ExternalInput")
    c2s_d = nc.dram_tensor("c2s", [K, 1], F32, kind="ExternalInput")
    c1r_d = nc.dram_tensor("c1row", [1, K], F32, kind="ExternalInput")
    out_d = nc.dram_tensor("out", [N, D], BF16, kind="ExternalOutput")

    with ExitStack() as ctx:
        tc = ctx.enter_context(tile.TileContext(nc))
        res = ctx.enter_context(tc.tile_pool(name="res", bufs=1))
        scr = ctx.enter_context(tc.tile_pool(name="scr", bufs=12))
        grp = ctx.enter_context(tc.tile_pool(name="grp", bufs=4))
        sml = ctx.enter_context(tc.tile_pool(name="sml", bufs=10))
        eqp = ctx.enter_context(tc.tile_pool(name="eqp", bufs=3))
        dgp = ctx.enter_context(tc.tile_pool(name="dgp", bufs=33))
        opool = ctx.enter_context(tc.tile_pool(name="op", bufs=3))
        xtp = ctx.enter_context(tc.tile_pool(name="xtp", bufs=3))

        # ---- residents & constants ----
        x_res = res.tile([128, NT, D], BF16)
        e_bf = res.tile([128, N], BF16)        # exp(z + c2 - C), bf16 (k, n)
        wagT_r = res.tile([128, DC, K], BF16)  # z lhsT chunks (d_sub, k)
        wbT_sb = res.tile([128, D], F32)       # (k, d)
        wbs_bf = res.tile([128, D], BF16)      # (k, d) * invZ
        wbT_bf = res.tile([128, D], BF16)      # (k, d) unscaled
        G1_sb = res.tile([128, K], F32)        # Gram of wbT (unscaled)
        G3_sb = res.tile([128, K], F32)
        G_bf = res.tile([128, K], BF16)        # Gram of wbs
        rhs_cols = res.tile([128, 2], BF16)    # [invZ | rowsum(wbs)/D]
        ones_bf = res.tile([128, 1], BF16)
        ones_full = res.tile([128, D], BF16)   # Pool-stats m1 multiplicand
        ident = res.tile([128, 128], F32)
        ident_bf = res.tile([128, 128], BF16)
        c2s_sb = res.tile([128, 1], F32)       # c2 - C_SHIFT
        c1_bf = res.tile([1, K], BF16)         # rowsum(W_ag) as a 1-part row
        eps_c = res.tile([128, 1], F32)
        bst = res.tile([128, NT, 3], F32)      # per-tile [S, p, sq] columns
        Zpart = res.tile([128, NW], F32)       # per-group exp-sum partials
        sstar = res.tile([128, NT], F32)
        bstar = res.tile([128, NT], F32)
        sd2 = res.tile([128, NT], F32)

        make_identity(nc, ident)
        nc.vector.tensor_copy(ident_bf, ident)
        nc.vector.memset(eps_c, LN_EPS)
        nc.vector.memset(ones_bf, 1.0)
        nc.vector.memset(ones_full, 1.0)
        nc.sync.dma_start(c2s_sb, c2s_d[:, :])
        c1_f32 = scr.tile([1, K], F32, tag="c1s")
        nc.sync.dma_start(c1_f32, c1r_d[:, :])
        nc.vector.tensor_copy(c1_bf, c1_f32)
        nc.sync.dma_start(wbT_sb, wbT_d[:, :])
        wag_stage = scr.tile([128, D], BF16, tag="scr")
        nc.sync.dma_start(
            wag_stage.rearrange("p (c k) -> p c k", c=DC),
            wagT_d.ap().rearrange("(c p) k -> p c k", p=128),
        )
        nc.vector.tensor_copy(
            wagT_r, wag_stage.rearrange("p (c k) -> p c k", c=DC)
        )
        nc.vector.tensor_copy(wbT_bf, wbT_sb)

        # Gram of unscaled wbT at kernel start (PE idle while x streams in);
        # G(wbs) = diag(invZ) G' diag(invZ) applied cheaply in MID.
        with tc.tile_pool(name="psG", bufs=1, space="PSUM") as psG:
            psGt = psG.tile([128, DC, 128], BF16, tag="gt")
            wbsT_stage = scr.tile([128, D], BF16, tag="wt")
            for c in range(DC):
                nc.tensor.transpose(
                    psGt[:, c, :], wbT_bf[:, c * 128:(c + 1) * 128], ident_bf
                )
            nc.vector.tensor_copy(
                wbsT_stage.rearrange("p (c k) -> p c k", c=DC), psGt
            )
            G_ps = psG.tile([128, K], F32, tag="g")
            wtv = wbsT_stage.rearrange("p (c k) -> p c k", c=DC)
            for c in range(DC):
                nc.tensor.matmul(
                    G_ps, wtv[:, c, :], wtv[:, c, :],
                    start=(c == 0), stop=(c == DC - 1),
                )
            nc.vector.tensor_copy(G1_sb, G_ps)

        import contextlib
        rep_ctx = tc.For_i(0, reps, 1) if reps > 1 else contextlib.nullcontext()
        with rep_ctx:
            # ====== PHASE A:  e = exp(W_ag @ x^T R + c1 (x) u + c2s) =========
            # R = diag(r) rides the transposes as the moving operand; the
            # mean correction enters z as rank-1 c1 (x) (-m r) matmuls.
            with tc.tile_pool(name="psA", bufs=5, space="PSUM") as psA, \
                 tc.tile_pool(name="psZ", bufs=2, space="PSUM") as psZ, \
                 tc.tile_pool(name="psU", bufs=1, space="PSUM") as psU:
                pending = []     # deferred (w, xT_g, uT_sb): z-mms lag the
                                 # transposes one wave so PE never stalls on
                                 # the PSUM->SBUF evicts

                def emit_z(w, xT_g, uT_sb):
                    z_ps = psZ.tile([128, 128 * WV], F32, tag="z")
                    for c in range(DC):
                        nc.tensor.matmul(
                            z_ps, wagT_r[:, c, :], xT_g[:, c, :],
                            start=(c == 0), stop=False,
                        )
                    for i in range(WV):
                        nc.tensor.matmul(
                            z_ps[:, i * 128:(i + 1) * 128], c1_bf,
                            uT_sb[:, i, :], start=False, stop=(i == WV - 1),
                            skip_group_check=True,
                        )
                    zsl = slice(w * 128 * WV, (w + 1) * 128 * WV)
                    nc.scalar.activation(
                        e_bf[:, zsl], z_ps, AF.Exp, bias=c2s_sb, scale=1.0,
                        accum_out=Zpart[:, w:w + 1],
                    )

                for w in range(NW):
                    js = [w * WV + i for i in range(WV)]
                    for h in range(WV // 2):
                        nc.sync.dma_start(
                            x_res[:, js[0] + 2 * h:js[0] + 2 * h + 2, :],
                            x_d.ap().rearrange("(t p) d -> p t d", p=128)[
                                :, js[0] + 2 * h:js[0] + 2 * h + 2, :],
                        )
                    mvw = sml.tile([128, WV, 2], F32, tag="mv")
                    m12 = sml.tile([128, WV, 2], F32, tag="m12")
                    for i, j in enumerate(js):
                        xj = x_res[:, j, :]
                        st = STAT_ENG[j]
                        if st == 'D':
                            stw = sml.tile([128, 2, 6], F32, tag="st")
                            nc.vector.bn_stats(stw[:, 0, :], xj[:, 0:512])
                            nc.vector.bn_stats(stw[:, 1, :], xj[:, 512:1024])
                            nc.vector.bn_aggr(mvw[:, i, :], stw)
                        elif st == 'A':
                            junk = scr.tile([128, D], BF16, tag="scr")
                            nc.scalar.activation(
                                junk, xj, AF.Identity,
                                accum_out=m12[:, i, 0:1])
                            nc.scalar.activation(
                                junk, xj, AF.Square,
                                accum_out=m12[:, i, 1:2])

                    # unify: mean/var for A/P tiles from m1/m2 (DVE smalls)
                    for i, j in enumerate(js):
                        if STAT_ENG[j] == 'D':
                            continue
                        mcol = mvw[:, i, 0:1]
                        nc.vector.tensor_scalar(
                            mcol, m12[:, i, 0:1], 1.0 / D, None, op0=OP.mult)
                        negmm = sml.tile([128, 1], F32, tag="nmm")
                        nc.vector.scalar_tensor_tensor(
                            negmm, mcol, -1.0, mcol, op0=OP.mult, op1=OP.mult)
                        nc.vector.scalar_tensor_tensor(
                            mvw[:, i, 1:2], m12[:, i, 1:2], 1.0 / D, negmm,
                            op0=OP.mult, op1=OP.add)
                    # r = 1/sqrt(var+eps); u = -mean*r (bf16, for rank-1)
                    sd_w = grp.tile([128, WV], F32, tag="sdw")
                    nc.scalar.activation(sd_w, mvw[:, :, 1], AF.Sqrt,
                                         bias=eps_c)
                    r_w = grp.tile([128, WV], F32, tag="rw")
                    nc.vector.reciprocal(r_w, sd_w)
                    u_w = grp.tile([128, WV], BF16, tag="uw")
                    nc.vector.scalar_tensor_tensor(
                        u_w, mvw[:, :, 0], -1.0, r_w, op0=OP.mult, op1=OP.mult)
                    # transpose each u column -> partition-0 rows (1, 128)
                    # so the rank-1 rhs sits at base partition 0
                    uT_ps = psU.tile([1, WV, 128], BF16, tag="ut")
                    for i in range(WV):
                        nc.tensor.transpose(uT_ps[:, i, :], u_w[:, i:i + 1],
                                            ident_bf)
                    uT_sb = grp.tile([1, WV, 128], BF16, tag="utsb")
                    nc.vector.tensor_copy(uT_sb, uT_ps)
                    # per-tile xr = x * r (the transpose datapath ignores the
                    # moving operand's values, so scaling must happen here)
                    xrs = []
                    for i, j in enumerate(js):
                        xr = scr.tile([128, D], BF16, tag="scr")
                        if XR_ENG[j] == 'P':
                            nc.gpsimd.tensor_scalar(
                                xr, x_res[:, j, :], r_w[:, i:i + 1], None,
                                op0=OP.mult)
                        elif XR_ENG[j] == 'A':
                            nc.scalar.activation(
                                xr, x_res[:, j, :], AF.Identity,
                                scale=r_w[:, i:i + 1])
                        else:
                            nc.vector.tensor_scalar(
                                xr, x_res[:, j, :], r_w[:, i:i + 1], None,
                                op0=OP.mult)
                        xrs.append(xr)
                    # transpose per tile -> PSUM, evict to xT_g
                    xT_g = xtp.tile([128, DC, 128 * WV], BF16, tag="xt")
                    for tt in range(WV):
                        psT = psA.tile([128, DC, 128], BF16, tag="psT")
                        for c in range(DC):
                            nc.tensor.transpose(
                                psT[:, c, :],
                                xrs[tt][:, c * 128:(c + 1) * 128],
                                ident_bf,
                            )
                        for hh in range(2):
                            u = (w * WV + tt) * 2 + hh
                            src = psT[:, 4 * hh:4 * hh + 4, :]
                            dst = xT_g[:, 4 * hh:4 * hh + 4,
                                       tt * 128:(tt + 1) * 128]
                            if EV_ENG[u] == 'A':
                                nc.scalar.activation(dst, src, AF.Copy)
                            elif EV_ENG[u] == 'D':
                                nc.vector.tensor_copy(dst, src)
                            else:
                                nc.sync.dma_start(dst, src)
                    pending.append((w, xT_g, uT_sb))
                    if len(pending) > 1:
                        emit_z(*pending.pop(0))
                for args in pending:
                    emit_z(*args)

            # =================== MID: softmax prep + Gram scaling =============
            Z_col = sml.tile([128, 1], F32, tag="Z")
            nc.vector.reduce_sum(Z_col, Zpart, axis=AX.X)
            invZ = sml.tile([128, 1], F32, tag="invZ")
            nc.vector.reciprocal(invZ, Z_col)
            nc.vector.tensor_scalar(wbs_bf, wbT_sb, invZ, None, op0=OP.mult)
            wrs = sml.tile([128, 1], F32, tag="wrs")
            nc.vector.reduce_sum(wrs, wbs_bf, axis=AX.X)
            wrs_s = sml.tile([128, 1], F32, tag="wrss")
            nc.vector.tensor_scalar(wrs_s, wrs, 1.0 / D, None, op0=OP.mult)
            nc.vector.tensor_copy(rhs_cols[:, 0:1], invZ)
            nc.vector.tensor_copy(rhs_cols[:, 1:2], wrs_s)

            # G(wbs) = diag(invZ) G' diag(invZ): row-scale, transpose, row-scale
            with tc.tile_pool(name="psG2", bufs=1, space="PSUM") as psG2:
                G2 = scr.tile([128, K], F32, tag="g2")
                nc.vector.tensor_scalar(G2, G1_sb, invZ, None, op0=OP.mult)
                G2_ps = psG2.tile([128, K], F32, tag="g2p")
                nc.tensor.transpose(G2_ps, G2, ident)
                nc.vector.tensor_copy(G3_sb, G2_ps)
                nc.vector.tensor_scalar(G_bf, G3_sb, invZ, None, op0=OP.mult)

            # =================== PHASE B ======================================
            with tc.tile_pool(name="psQ", bufs=1, space="PSUM") as psQ, \
                 tc.tile_pool(name="psC", bufs=1, space="PSUM") as psC, \
                 tc.tile_pool(name="psB", bufs=3, space="PSUM") as psB:
                stage = []   # (j, diag_j, rhs_x) staged for the B2 sweep
                for g in range(NG):
                    gs = slice(g * GS, (g + 1) * GS)
                    e_g = e_bf[:, g * 512:(g + 1) * 512]
                    Q_ps = psQ.tile([128, 512], F32, tag="q")
                    nc.tensor.matmul(Q_ps, G_bf, e_g, start=True, stop=True)
                    eq = eqp.tile([128, 512], BF16, tag="eq")
                    nc.vector.tensor_mul(eq, e_g, Q_ps)
                    cps = psC.tile([128, GS, 3], F32, tag="c")
                    for jj in range(GS):
                        j = g * GS + jj
                        nc.tensor.matmul(
                            cps[:, jj, 0:2], e_bf[:, j * 128:(j + 1) * 128],
                            rhs_cols, start=True, stop=True,
                        )
                        nc.tensor.matmul(
                            cps[:, jj, 2:3], eq[:, jj * 128:(jj + 1) * 128],
                            ones_bf, start=True, stop=True,
                        )
                    nc.vector.tensor_copy(bst[:, gs, :], cps)

                    # epilogue coefficients for this group: (128, GS) ops
                    S_g = bst[:, gs, 0]
                    p_g = bst[:, gs, 1]
                    sq_g = bst[:, gs, 2]
                    Sp = grp.tile([128, GS], F32, tag="Sp")
                    nc.vector.tensor_scalar(Sp, S_g, 1e-9, None, op0=OP.add)
                    u1 = grp.tile([128, GS], F32, tag="u1")
                    nc.vector.scalar_tensor_tensor(u1, Sp, LN_EPS, Sp,
                                                   op0=OP.mult, op1=OP.mult)
                    q3 = grp.tile([128, GS], F32, tag="q3")
                    nc.vector.scalar_tensor_tensor(q3, sq_g, 1.0 / D, u1,
                                                   op0=OP.mult, op1=OP.add)
                    pp = grp.tile([128, GS], F32, tag="pp")
                    nc.vector.tensor_mul(pp, p_g, p_g)
                    u4n = grp.tile([128, GS], F32, tag="u4n")
                    nc.vector.tensor_sub(u4n, pp, q3)     # p^2 - (q + eps Sp^2)
                    nc.scalar.activation(sd2[:, gs], u4n, AF.Sqrt, bias=0.0,
                                         scale=-1.0)
                    nc.vector.reciprocal(sstar[:, gs], sd2[:, gs])
                    nc.vector.scalar_tensor_tensor(bstar[:, gs], p_g, -1.0,
                                                   sstar[:, gs],
                                                   op0=OP.mult, op1=OP.mult)

                    # stage the per-tile matmul operands (diag on Pool, xp on
                    # DVE) so the B2 sweep below runs dense on PE
                    for jj in range(GS):
                        j = g * GS + jj
                        diag_j = dgp.tile([128, 128], BF16, tag="dg")
                        nc.gpsimd.tensor_scalar(
                            diag_j, ident_bf, sd2[:, j:j + 1], None,
                            op0=OP.mult,
                        )
                        if EPI_ENG[j] == 'A':
                            rhs_x = x_res[:, j, :]
                        else:
                            # fold b* into the residual: diag(sd2)@(x + b*)
                            # == sd2*x - p, so epilogue is s* * max(psum, 0)
                            xp = scr.tile([128, D], BF16, tag="scr")
                            nc.vector.tensor_scalar(
                                xp, x_res[:, j, :], bstar[:, j:j + 1], None,
                                op0=OP.add,
                            )
                            rhs_x = xp
                        stage.append((j, diag_j, rhs_x))

                # ---- B2: dense h3 + residual + epilogue sweep ----
                for j, diag_j, rhs_x in stage:
                    e_sl = e_bf[:, j * 128:(j + 1) * 128]
                    epi = EPI_ENG[j]
                    h3_ps = psB.tile([128, D], F32, tag="h3")
                    for hh in range(2):
                        sl = slice(hh * 512, (hh + 1) * 512)
                        nc.tensor.matmul(
                            h3_ps[:, sl], e_sl, wbs_bf[:, sl],
                            start=True, stop=False,
                        )
                        nc.tensor.matmul(
                            h3_ps[:, sl], diag_j, rhs_x[:, sl],
                            start=False, stop=True, skip_group_check=True,
                        )
                    o_sb = opool.tile([128, D], BF16, tag="o")
                    if epi == 'A':
                        nc.scalar.activation(
                            o_sb, h3_ps, AF.Relu,
                            bias=bstar[:, j:j + 1], scale=sstar[:, j:j + 1],
                        )
                    else:
                        nc.vector.tensor_scalar(
                            o_sb, h3_ps, 0.0, sstar[:, j:j + 1],
                            op0=OP.max, op1=OP.mult,
                        )
                    nc.sync.dma_start(out_d[j * 128:(j + 1) * 128, :], o_sb)

    nc.compile()
    return nc


def _to_bf16(a):
    import ml_dtypes
    return np.asarray(a, dtype=ml_dtypes.bfloat16)


def _host_precompute(inputs):
    f64 = np.float64
    w_in = np.asarray(inputs["w_in"], f64)
    b_in = np.asarray(inputs["b_in"], f64)
    w0 = np.asarray(inputs["w0"], f64)
    w1 = np.asarray(inputs["w1"], f64)
    w_out = np.asarray(inputs["w_out"], f64)
    ln_g = np.asarray(inputs["ln_g"], f64)
    ln_b = np.asarray(inputs["ln_b"], f64)
    oln_g = np.asarray(inputs["oln_g"], f64)
    oln_b = np.asarray(inputs["oln_b"], f64)

    W_a = w0 @ w_in                     # (K, D)
    W_ag = W_a * ln_g[None, :]          # (K, D)
    c2 = W_a @ ln_b + w0 @ b_in         # (K,)
    W_b = w_out @ w1                    # (D, K)

    # the on-device output LN applies no gamma/beta; require trivial ones
    # (true for this module). Fail loudly otherwise.
    assert np.allclose(oln_g, 1.0) and np.allclose(oln_b, 0.0), (
        "kernel fast path requires oln_g == 1 and oln_b == 0"
    )

    return {
        "w_agT": _to_bf16(np.ascontiguousarray(W_ag.T)),             # (D, K)
        "w_bT": np.ascontiguousarray(W_b.T.astype(np.float32)),      # (K, D)
        "c2s": (c2 - C_SHIFT).astype(np.float32).reshape(K, 1),
        "c1row": W_ag.sum(axis=1).astype(np.float32).reshape(1, K),
    }


def kernel(**inputs) -> np.ndarray:
    if "nc" not in _CACHE:
        _CACHE["nc"] = _build()
    nc = _CACHE["nc"]

    shared = _host_precompute(inputs)
    x = np.asarray(inputs["x"], np.float32)
    in_maps = [{"x": _to_bf16(x[b]), **shared} for b in range(B)]
    res = run_bass_kernel_spmd(nc, in_maps, list(range(B)))
    out = np.stack(
        [np.asarray(res.results[b]["out"]).astype(np.float32)
         for b in range(B)],
        axis=0,
    )
    return out


if __name__ == "__main__":
    rng = np.random.default_rng(0)
    demo = {
        "x": rng.standard_normal((B, N, D)).astype(np.float32),
        "ln_g": np.ones(D, np.float32),
        "ln_b": np.zeros(D, np.float32),
        "w_in": (rng.standard_normal((D, D)) * np.sqrt(2 / D)).astype(np.float32),
        "b_in": np.zeros(D, np.float32),
        "w0": (rng.standard_normal((K, D)) * np.sqrt(2 / K)).astype(np.float32),
        "w1": (rng.standard_normal((D, K)) * np.sqrt(2 / D)).astype(np.float32),
        "w_out": (rng.standard_normal((D, D)) * np.sqrt(2 / D)).astype(np.float32),
        "oln_g": np.ones(D, np.float32),
        "oln_b": np.zeros(D, np.float32),
    }
    out = kernel(**demo)
    print("kernel ran:", out.shape, out.dtype)
